# revision 10
# baseline (speedup 1.0000x reference)
"""GNN message-passing kernel for Trainium2 (8 NeuronCores, SPMD).

Reference computation:
    msg  = x[src] * edge_weight[:, None]
    agg  = segment_sum(msg, dst, N) / max(segment_sum(1, dst, N), 1)
    out  = x + alpha * (agg @ W.T + b)

Sharding: nodes are sharded across 8 cores by contiguous ranges; edges are
partitioned by dst so scatter-adds stay local; x is replicated to every
core's DRAM so the src-row gather is always local.

Per core, dst nodes are grouped into 128-node blocks. A block's incoming
edges are processed in chunks of 128 edges: a dma_gather (InstDMAGatherAnt)
fetches the 128 src rows in bf16 (one per partition), a single DVE
tensor_scalar builds the weighted one-hot selection matrix
Sw[e, j] = w[e]*(slot[e]==j) in bf16, and the tensor engine accumulates
aggT[f, n] += Xg[e, f]^T @ Sw[e, n] in PSUM (bf16 matmul = 1 cyc/row vs 4
for fp32). A second bf16 matmul applies W^T, the scalar engine scales by
alpha/deg, and DVE adds the residual (bias pre-folded into it on host).

dma_gather uses int16 indices, so src space is split into 4 buckets of
<= 32768 rows; each chunk's edges come from a single (block, bucket) group.
Gathers are batched (one dma_gather per bucket per ~12-block batch) to
amortize prep/drain pipelining; calls are capped at 1024 indices (ucode
limit) and the SWDGE ring holds 4096 descriptors so several calls can be
in flight.

All 8 cores run one shared program: each core orders its blocks by
descending chunk count and the program uses the per-position max, so the
control flow is identical and only the data differs.
"""

import numpy as np
from ml_dtypes import bfloat16

P = 128
NCORES = 8
NBUCK = 4

# set by test harness for profiling; grading leaves these defaults
TRACE = False
LAST_RESULTS = None
GATHER_BLOCKS = 12      # block-slots per gather batch
REPEAT = 1              # repeat program body (timing experiments only)
MAX_GATHER_CHUNKS = 8   # chunks (x128 idx) per dma_gather call (1024-idx ucode cap)
DMA_SCRATCH = 65536     # SWDGE descriptor ring: 4096 descs (4 calls in flight)


# Canonical per-block chunk templates. Aligning (nearly) every block to the
# same bucket->chunk-count vector means the cross-core per-position max adds
# almost nothing, so the shared schedule stays near the per-core optimum.
TMPL_A = np.array([2, 1, 2, 2])   # 7 chunks; feasible for ~94% of blocks
TMPL_B = np.array([2, 2, 2, 2])   # 8 chunks


def _rebalance_buckets(core, blk, src, starts, NBLK):
    """Assign each edge to a src-index bucket window, using the overlap
    between adjacent 32768-row windows to fit each (core, block) into a
    canonical chunk template. Returns (bucket id per edge, class rank per
    (core, block)): rank 0 = irregular, 1 = template B, 2 = template A."""
    WIN = 32768
    nz = len(starts) - 1
    buck = np.zeros(src.shape[0], np.int8)
    for q in range(1, len(starts)):
        buck[src >= starts[q - 1] + WIN] = q
    # movable edges: in the overlap of window q and q+1 (natively in q)
    zone = np.full(src.shape[0], -1, np.int8)
    for z in range(nz):
        m = (src >= starts[z + 1]) & (src <= starts[z] + WIN - 1)
        zone[m] = z
    rank = np.zeros((NCORES, NBLK), np.int64)
    gkey = core * NBLK + blk
    order = np.argsort(gkey, kind="stable")
    bounds = np.searchsorted(gkey[order], np.arange(NCORES * NBLK + 1))
    for g in range(NCORES * NBLK):
        sl = order[bounds[g]:bounds[g + 1]]
        if sl.size == 0:
            rank[g // NBLK, g % NBLK] = 2
            continue
        zs = zone[sl]
        bq = buck[sl]
        base = np.bincount(bq[np.logical_or(zs < 0, bq > zs)],
                           minlength=nz + 1).astype(np.int64)
        mov = np.bincount(zs[np.logical_and(zs >= 0, bq == zs)], minlength=nz)
        for caps, r in ((TMPL_A * 128, 2), (TMPL_B * 128, 1)):
            # waterfill left->right, pushing right only when forced
            t = np.zeros(nz, np.int64)
            t_prev = 0
            ok = True
            for q in range(nz + 1):
                inflow = base[q] + t_prev + (mov[q] if q < nz else 0)
                if q < nz:
                    t[q] = min(max(inflow - caps[q], 0), mov[q])
                    t_prev = t[q]
                    inflow -= t[q]
                if inflow > caps[q]:
                    ok = False
                    break
            if ok:
                rank[g // NBLK, g % NBLK] = r
                for z in range(nz):
                    if t[z]:
                        zi = sl[np.logical_and(zs == z, bq == z)]
                        buck[zi[:t[z]]] = z + 1
                break
    return buck, rank


def _preprocess(x, src, dst, w):
    N, D = x.shape
    E = src.shape[0]
    SH = -(-N // NCORES)          # nodes per core shard
    NBLK = -(-SH // P)            # 128-node blocks per core
    SHP = NBLK * P                # padded shard size
    WIN = 32768
    starts = [round(q * (N - WIN) / (NBUCK - 1)) for q in range(NBUCK)]
    assert starts[-1] + WIN >= N

    core = dst // SH
    rel = dst - core * SH
    blk = rel // P
    slot = rel % P
    buck, rank = _rebalance_buckets(core, blk, src, starts, NBLK)
    starts_a = np.asarray(starts, np.int64)

    # per (core, block, bucket) edge counts -> chunk counts
    key = (core * NBLK + blk) * NBUCK + buck
    counts = np.bincount(key, minlength=NCORES * NBLK * NBUCK)
    counts = counts.reshape(NCORES, NBLK, NBUCK)
    chunks = -(-counts // P)                                # [NC, NBLK, NBUCK]
    # template-class blocks use the fixed canonical vector so the shared
    # cross-core max schedule adds (almost) no padding
    chunks = np.where((rank == 2)[:, :, None], TMPL_A[None, None, :], chunks)
    chunks = np.where((rank == 1)[:, :, None], TMPL_B[None, None, :], chunks)
    tot = chunks.sum(axis=2)
    # blocks with no edges still need one (dummy) chunk to init PSUM
    empty = counts.sum(axis=2) == 0
    chunks[:, :, 0] = np.where(empty, 1, chunks[:, :, 0])
    chunks[:, :, 1:] = np.where(empty[:, :, None], 0, chunks[:, :, 1:])
    tot = chunks.sum(axis=2)

    # order: irregular blocks first (desc total), then template B, then A,
    # then empty - identical vectors align across cores at each position
    sort_key = rank * 10_000 - tot + np.where(empty, 1_000_000, 0)
    perm = np.argsort(sort_key, axis=1, kind="stable")      # block order per core
    # shared schedule: per (slot-position, bucket) max chunk count over cores
    sorted_chunks = np.take_along_axis(chunks, perm[:, :, None], axis=1)
    NCH4 = sorted_chunks.max(axis=0)                        # [NBLK, NBUCK]

    # global chunk order: batches of GB slots; within a batch buckets are
    # contiguous (one dma_gather per bucket): for b: for q: for s in b: chunks
    GB = GATHER_BLOCKS
    K_of = np.zeros((NBLK, NBUCK), np.int64)                # chunk start of (s, q)
    batches = []   # (s0, s1, gstart, [(q, off_in_batch, nchunks)])
    kg = 0
    for s0 in range(0, NBLK, GB):
        s1 = min(s0 + GB, NBLK)
        gstart = kg
        calls = []
        for q in range(NBUCK):
            off = kg - gstart
            n_q = 0
            for s in range(s0, s1):
                K_of[s, q] = kg
                kg += int(NCH4[s, q])
                n_q += int(NCH4[s, q])
            calls.append((q, off, n_q))
        batches.append((s0, s1, gstart, calls))
    C_total = kg

    inv_perm = np.empty_like(perm)
    np.put_along_axis(
        inv_perm, perm,
        np.broadcast_to(np.arange(NBLK), (NCORES, NBLK)).copy(), axis=1)

    # edge placement: flat position = K_of[s, q]*128 + rank within group
    order = np.argsort(key, kind="stable")
    grp_start = np.zeros(NCORES * NBLK * NBUCK, np.int64)
    grp_start[1:] = np.cumsum(counts.ravel())[:-1]
    pos_in_grp = np.arange(E) - grp_start[key[order]]
    co = core[order]
    s_of = inv_perm[co, blk[order]]
    padpos = K_of[s_of, buck[order]] * P + pos_in_grp

    idx_a = np.zeros((NCORES, C_total * P), np.int16)
    slot_a = np.full((NCORES, C_total * P), 200.0, np.float32)
    w_a = np.zeros((NCORES, C_total * P), np.float32)
    idx_a[co, padpos] = (src[order] - starts_a[buck[order]]).astype(np.int16)
    slot_a[co, padpos] = slot[order].astype(np.float32)
    w_a[co, padpos] = w[order]

    # dma_gather index wrap: index i -> [i % 16, i // 16], replicated to 128
    idx16 = idx_a.reshape(NCORES, C_total * 8, 16).transpose(0, 2, 1)
    idx16 = np.ascontiguousarray(
        np.broadcast_to(idx16[:, None, :, :], (NCORES, 8, 16, C_total * 8))
        .reshape(NCORES, P, C_total * 8))
    # per-chunk columns for tensor_scalar scalars (bf16)
    slot_t = np.ascontiguousarray(
        slot_a.reshape(NCORES, C_total, P).transpose(0, 2, 1))
    w_t = np.ascontiguousarray(w_a.reshape(NCORES, C_total, P).transpose(0, 2, 1))

    deg = np.bincount(dst, minlength=N).astype(np.float32)
    n_core = np.minimum(SH, N - np.arange(NCORES) * SH)
    ids = (np.arange(NCORES)[:, None, None] * SH
           + perm[:, :, None] * P + np.arange(P)[None, None, :])  # [NC, NBLK, P]
    valid = (perm[:, :, None] * P
             + np.arange(P)[None, None, :]) < n_core[:, None, None]
    ids_c = np.where(valid, ids, 0)

    xr = np.zeros((NCORES, NBLK, P, D), np.float32)
    xr[valid] = x[ids_c[valid]]
    xr = xr.reshape(NCORES, SHP, D)

    dt = np.zeros((NCORES, NBLK, P), np.float32)
    dt[valid] = deg[ids_c[valid]]
    deg_t = np.ascontiguousarray(dt.transpose(0, 2, 1))     # [NC, 128, NBLK]

    return dict(
        N=N, D=D, SH=SH, NBLK=NBLK, SHP=SHP, starts=starts, C_total=C_total,
        NCH4=NCH4, K_of=K_of, batches=batches,
        idx16=idx16, slot_t=slot_t, w_t=w_t,
        xr=xr, deg_t=deg_t, ids=ids, valid=valid,
    )


def _build_program(pre, alpha):
    import concourse.bacc as bacc
    import concourse.bass as bass
    import concourse.tile as tile
    from concourse import mybir

    f32 = mybir.dt.float32
    bf16 = mybir.dt.bfloat16
    eq = mybir.AluOpType.is_equal
    mult = mybir.AluOpType.mult
    mx = mybir.AluOpType.max

    N, NBLK, SHP, starts = pre["N"], pre["NBLK"], pre["SHP"], pre["starts"]
    C_total, NCH4, K_of = pre["C_total"], pre["NCH4"], pre["K_of"]
    batches = pre["batches"]

    nc = bacc.Bacc(None, target_bir_lowering=False,
                   dynamic_dma_scratch_size=DMA_SCRATCH)
    x_d = nc.dram_tensor("xh", [N, P], bf16, kind="ExternalInput")
    idx_d = nc.dram_tensor("idx16", [P, C_total * 8], mybir.dt.int16,
                           kind="ExternalInput")
    slot_d = nc.dram_tensor("slot", [P, C_total], f32, kind="ExternalInput")
    wg_d = nc.dram_tensor("wg", [P, C_total], f32, kind="ExternalInput")
    xr_d = nc.dram_tensor("xr", [SHP, P], bf16, kind="ExternalInput")
    deg_d = nc.dram_tensor("deg", [P, NBLK], f32, kind="ExternalInput")
    wt_d = nc.dram_tensor("wt", [P, P], bf16, kind="ExternalInput")
    iota_d = nc.dram_tensor("iota", [P, P], bf16, kind="ExternalInput")
    y_d = nc.dram_tensor("y", [SHP, P], bf16, kind="ExternalOutput")

    with tile.TileContext(nc) as tc:
        with (
            tc.tile_pool(name="const", bufs=1) as cpool,
            tc.tile_pool(name="gather", bufs=2) as gpool,
            tc.tile_pool(name="sw", bufs=6) as swpool,
            tc.tile_pool(name="agg", bufs=3) as aggpool,
            tc.tile_pool(name="xr", bufs=2) as xrpool,
            tc.tile_pool(name="ot", bufs=2) as otpool,
            tc.tile_pool(name="ps1", bufs=2, space="PSUM") as ps1,
            tc.tile_pool(name="ps2", bufs=2, space="PSUM") as ps2,
        ):
            idx_s = cpool.tile([P, C_total * 8], mybir.dt.int16)
            nc.sync.dma_start(out=idx_s[:], in_=idx_d[:, :])
            slot_s = cpool.tile([P, C_total], f32)
            nc.sync.dma_start(out=slot_s[:], in_=slot_d[:, :])
            w_s = cpool.tile([P, C_total], f32)
            nc.sync.dma_start(out=w_s[:], in_=wg_d[:, :])
            wt_s = cpool.tile([P, P], bf16)
            nc.sync.dma_start(out=wt_s[:], in_=wt_d[:, :])
            iota_s = cpool.tile([P, P], bf16)
            nc.sync.dma_start(out=iota_s[:], in_=iota_d[:, :])

            deg_s = cpool.tile([P, NBLK], f32)
            nc.sync.dma_start(out=deg_s[:], in_=deg_d[:, :])
            invd_s = cpool.tile([P, NBLK], f32)
            nc.vector.tensor_scalar(
                out=invd_s[:], in0=deg_s[:], scalar1=1.0, scalar2=None, op0=mx)
            nc.vector.reciprocal(invd_s[:], invd_s[:])
            nc.scalar.mul(invd_s[:], invd_s[:], float(alpha))

            for _rep in range(REPEAT):
              for (s0, s1, gstart, calls) in batches:
                nb = s1 - s0
                M = sum(n_q for (_, _, n_q) in calls)

                G = gpool.tile([P, M, P], bf16)
                for (q, off, n_q) in calls:
                    w0 = starts[q]
                    w1 = min(w0 + 32768, N)
                    for c0 in range(0, n_q, MAX_GATHER_CHUNKS):
                        n_c = min(MAX_GATHER_CHUNKS, n_q - c0)
                        o = off + c0
                        i0 = (gstart + o) * P    # global index position
                        nc.gpsimd.dma_gather(
                            out_ap=G[:, o:o + n_c, :],
                            in_ap=x_d[w0:w1, :],
                            idxs_ap=idx_s[:, i0 // 16:(i0 + n_c * P) // 16],
                            num_idxs=n_c * P,
                            num_idxs_reg=n_c * P,
                            elem_size=P,
                        )

                xrt = xrpool.tile([P, nb, P], bf16)
                nc.sync.dma_start(
                    out=xrt[:],
                    in_=xr_d[s0 * P:s1 * P, :].rearrange(
                        "(nb p) d -> p nb d", p=P),
                )
                ot = otpool.tile([P, nb, P], bf16)

                for s in range(s0, s1):
                    nch = int(NCH4[s].sum())
                    p1 = ps1.tile([P, P], f32)
                    ci = 0
                    for q in range(NBUCK):
                        for c in range(int(NCH4[s, q])):
                            k = int(K_of[s, q]) + c
                            sw = swpool.tile([P, P], bf16)
                            nc.vector.tensor_scalar(
                                out=sw[:], in0=iota_s[:],
                                scalar1=slot_s[:, k:k + 1],
                                scalar2=w_s[:, k:k + 1],
                                op0=eq, op1=mult,
                            )
                            nc.tensor.matmul(
                                p1[:], lhsT=G[:, k - gstart, :], rhs=sw[:],
                                start=(ci == 0), stop=(ci == nch - 1),
                            )
                            ci += 1
                    aggT = aggpool.tile([P, P], bf16)
                    nc.scalar.mul(aggT[:], p1[:], 1.0)
                    p2 = ps2.tile([P, P], f32)
                    nc.tensor.matmul(
                        p2[:], lhsT=aggT[:], rhs=wt_s[:], start=True, stop=True)
                    j = s - s0
                    nc.scalar.mul(ot[:, j, :], p2[:], invd_s[:, s:s + 1])
                    nc.vector.tensor_add(ot[:, j, :], ot[:, j, :], xrt[:, j, :])

                nc.sync.dma_start(
                    out=y_d[s0 * P:s1 * P, :].rearrange(
                        "(nb p) d -> p nb d", p=P),
                    in_=ot[:],
                )

    nc.compile()
    return nc


def kernel(**inputs):
    global LAST_RESULTS
    x = np.ascontiguousarray(np.asarray(inputs["x"], dtype=np.float32))
    ei = np.asarray(inputs["edge_index"])
    w = np.ascontiguousarray(np.asarray(inputs["edge_weight"], dtype=np.float32))
    W = np.asarray(inputs["W"], dtype=np.float32)
    b = np.asarray(inputs["b"], dtype=np.float32)
    alpha = float(np.asarray(inputs["alpha"]))
    src = ei[0].astype(np.int64)
    dst = ei[1].astype(np.int64)

    pre = _preprocess(x, src, dst, w)
    N, D = pre["N"], pre["D"]
    assert D == P

    nc = _build_program(pre, alpha)

    xh = np.ascontiguousarray(x.astype(bfloat16))
    wt = np.ascontiguousarray(W.T.astype(bfloat16))
    iota = np.ascontiguousarray(
        np.broadcast_to(np.arange(P, dtype=np.float32), (P, P))).astype(bfloat16)
    # residual with bias pre-folded: xr' = x[ids] + alpha*b
    xr2 = pre["xr"] + (alpha * b.astype(np.float32))[None, None, :]

    in_maps = []
    for c in range(NCORES):
        in_maps.append({
            "xh": xh,
            "idx16": pre["idx16"][c],
            "slot": pre["slot_t"][c],
            "wg": pre["w_t"][c],
            "xr": np.ascontiguousarray(xr2[c].astype(bfloat16)),
            "deg": pre["deg_t"][c],
            "wt": wt,
            "iota": iota,
        })

    global LAST_NC, LAST_IN_MAPS, LAST_PRE
    LAST_NC, LAST_IN_MAPS, LAST_PRE = nc, in_maps, pre

    from concourse.bass_utils import run_bass_kernel_spmd
    kw = {"trace": True} if TRACE else {}
    res = run_bass_kernel_spmd(
        nc, in_maps, core_ids=list(range(NCORES)), **kw)
    LAST_RESULTS = res

    out = np.empty((N, P), np.float32)
    NBLK = pre["NBLK"]
    valid = pre["valid"]
    ids = pre["ids"]
    for c in range(NCORES):
        y = np.asarray(res.results[c]["y"]).astype(np.float32).reshape(NBLK, P, P)
        out[ids[c][valid[c]]] = y[valid[c]]
    return out


# revision 15
# speedup vs baseline: 1.2893x; 1.2893x over previous
"""GNN message-passing kernel for Trainium2 (8 NeuronCores, SPMD).

Reference computation:
    msg  = x[src] * edge_weight[:, None]
    agg  = segment_sum(msg, dst, N) / max(segment_sum(1, dst, N), 1)
    out  = x + alpha * (agg @ W.T + b)

Sharding: nodes are sharded across 8 cores by contiguous ranges; edges are
partitioned by dst so scatter-adds stay local; x is replicated to every
core's DRAM so the src-row gather is always local.

Per core, dst nodes are grouped into 128-node blocks. A block's incoming
edges are processed in chunks of 128 edges: a dma_gather (InstDMAGatherAnt)
fetches the 128 src rows in bf16 (one per partition), a single DVE
tensor_scalar builds the weighted one-hot selection matrix
Sw[e, j] = w[e]*(slot[e]==j) in bf16, and the tensor engine accumulates
aggT[f, n] += Xg[e, f]^T @ Sw[e, n] in PSUM (bf16 matmul = 1 cyc/row vs 4
for fp32). A second bf16 matmul applies W^T, the scalar engine scales by
alpha/deg, and DVE adds the residual (bias pre-folded into it on host).

dma_gather uses int16 indices, so src space is split into 4 buckets of
<= 32768 rows; each chunk's edges come from a single (block, bucket) group.
Gathers are batched (one dma_gather per bucket per ~12-block batch) to
amortize prep/drain pipelining; calls are capped at 1024 indices (ucode
limit) and the SWDGE ring holds 4096 descriptors so several calls can be
in flight.

All 8 cores run one shared program: each core orders its blocks by
descending chunk count and the program uses the per-position max, so the
control flow is identical and only the data differs.
"""

import numpy as np
from ml_dtypes import bfloat16

P = 128
NCORES = 8
NBUCK = 4

# set by test harness for profiling; grading leaves these defaults
TRACE = False
LAST_RESULTS = None
GATHER_BLOCKS = 12      # block-slots per gather batch
REPEAT = 1              # repeat program body (timing experiments only)
ABLATE = ""             # "", "gather_only", "no_gather" (timing experiments)
MAX_GATHER_CHUNKS = 8   # chunks (x128 idx) per dma_gather call (1024-idx ucode cap)
DMA_SCRATCH = 65536     # SWDGE descriptor ring: 4096 descs (4 calls in flight)


# Canonical per-block chunk templates. Aligning (nearly) every block to the
# same bucket->chunk-count vector means the cross-core per-position max adds
# almost nothing, so the shared schedule stays near the per-core optimum.
TMPL_A = np.array([2, 1, 2, 2])   # 7 chunks; feasible for ~94% of blocks
TMPL_B = np.array([2, 2, 2, 2])   # 8 chunks


def _rebalance_buckets(core, blk, src, starts, NBLK):
    """Assign each edge to a src-index bucket window, using the overlap
    between adjacent 32768-row windows to fit each (core, block) into a
    canonical chunk template. Returns (bucket id per edge, class rank per
    (core, block)): rank 0 = irregular, 1 = template B, 2 = template A."""
    WIN = 32768
    nz = len(starts) - 1
    buck = np.zeros(src.shape[0], np.int8)
    for q in range(1, len(starts)):
        buck[src >= starts[q - 1] + WIN] = q
    # movable edges: in the overlap of window q and q+1 (natively in q)
    zone = np.full(src.shape[0], -1, np.int8)
    for z in range(nz):
        m = (src >= starts[z + 1]) & (src <= starts[z] + WIN - 1)
        zone[m] = z
    rank = np.zeros((NCORES, NBLK), np.int64)
    gkey = core * NBLK + blk
    order = np.argsort(gkey, kind="stable")
    bounds = np.searchsorted(gkey[order], np.arange(NCORES * NBLK + 1))
    for g in range(NCORES * NBLK):
        sl = order[bounds[g]:bounds[g + 1]]
        if sl.size == 0:
            rank[g // NBLK, g % NBLK] = 2
            continue
        zs = zone[sl]
        bq = buck[sl]
        base = np.bincount(bq[np.logical_or(zs < 0, bq > zs)],
                           minlength=nz + 1).astype(np.int64)
        mov = np.bincount(zs[np.logical_and(zs >= 0, bq == zs)], minlength=nz)
        for caps, r in ((TMPL_A * 128, 2), (TMPL_B * 128, 1)):
            # waterfill left->right, pushing right only when forced
            t = np.zeros(nz, np.int64)
            t_prev = 0
            ok = True
            for q in range(nz + 1):
                inflow = base[q] + t_prev + (mov[q] if q < nz else 0)
                if q < nz:
                    t[q] = min(max(inflow - caps[q], 0), mov[q])
                    t_prev = t[q]
                    inflow -= t[q]
                if inflow > caps[q]:
                    ok = False
                    break
            if ok:
                rank[g // NBLK, g % NBLK] = r
                for z in range(nz):
                    if t[z]:
                        zi = sl[np.logical_and(zs == z, bq == z)]
                        buck[zi[:t[z]]] = z + 1
                break
    return buck, rank


def _preprocess(x, src, dst, w):
    N, D = x.shape
    E = src.shape[0]
    SH = -(-N // NCORES)          # nodes per core shard
    NBLK = -(-SH // P)            # 128-node blocks per core
    SHP = NBLK * P                # padded shard size
    WIN = 32768
    starts = [round(q * (N - WIN) / (NBUCK - 1)) for q in range(NBUCK)]
    assert starts[-1] + WIN >= N

    core = dst // SH
    rel = dst - core * SH
    blk = rel // P
    slot = rel % P
    buck, rank = _rebalance_buckets(core, blk, src, starts, NBLK)
    starts_a = np.asarray(starts, np.int64)

    # per (core, block, bucket) edge counts -> chunk counts
    key = (core * NBLK + blk) * NBUCK + buck
    counts = np.bincount(key, minlength=NCORES * NBLK * NBUCK)
    counts = counts.reshape(NCORES, NBLK, NBUCK)
    chunks = -(-counts // P)                                # [NC, NBLK, NBUCK]
    # template-class blocks use the fixed canonical vector so the shared
    # cross-core max schedule adds (almost) no padding
    chunks = np.where((rank == 2)[:, :, None], TMPL_A[None, None, :], chunks)
    chunks = np.where((rank == 1)[:, :, None], TMPL_B[None, None, :], chunks)
    tot = chunks.sum(axis=2)
    # blocks with no edges still need one (dummy) chunk to init PSUM
    empty = counts.sum(axis=2) == 0
    chunks[:, :, 0] = np.where(empty, 1, chunks[:, :, 0])
    chunks[:, :, 1:] = np.where(empty[:, :, None], 0, chunks[:, :, 1:])
    tot = chunks.sum(axis=2)

    # order: irregular blocks first (desc total), then template B, then A,
    # then empty - identical vectors align across cores at each position
    sort_key = rank * 10_000 - tot + np.where(empty, 1_000_000, 0)
    perm = np.argsort(sort_key, axis=1, kind="stable")      # block order per core
    # shared schedule: per (slot-position, bucket) max chunk count over cores
    sorted_chunks = np.take_along_axis(chunks, perm[:, :, None], axis=1)
    NCH4 = sorted_chunks.max(axis=0)                        # [NBLK, NBUCK]

    # global chunk order: batches of GB slots; within a batch buckets are
    # contiguous (one dma_gather per bucket): for b: for q: for s in b: chunks
    GB = GATHER_BLOCKS
    K_of = np.zeros((NBLK, NBUCK), np.int64)                # chunk start of (s, q)
    batches = []   # (s0, s1, gstart, [(q, s, off_in_batch, nchunks)])
    call_sq = []   # call order -> (s, q)
    kg = 0
    for s0 in range(0, NBLK, GB):
        s1 = min(s0 + GB, NBLK)
        gstart = kg
        groups = []
        for q in range(NBUCK):
            for s in range(s0, s1):
                K_of[s, q] = kg
                nch = int(NCH4[s, q])
                if nch:
                    groups.append((q, s, kg - gstart, nch))
                    call_sq.append((s, q))
                kg += nch
        batches.append((s0, s1, gstart, groups))
    C_total = kg
    NCALLS = len(call_sq)

    inv_perm = np.empty_like(perm)
    np.put_along_axis(
        inv_perm, perm,
        np.broadcast_to(np.arange(NBLK), (NCORES, NBLK)).copy(), axis=1)

    # edge placement: flat position = K_of[s, q]*128 + rank within group
    order = np.argsort(key, kind="stable")
    grp_start = np.zeros(NCORES * NBLK * NBUCK, np.int64)
    grp_start[1:] = np.cumsum(counts.ravel())[:-1]
    pos_in_grp = np.arange(E) - grp_start[key[order]]
    co = core[order]
    s_of = inv_perm[co, blk[order]]
    padpos = K_of[s_of, buck[order]] * P + pos_in_grp

    idx_a = np.zeros((NCORES, C_total * P), np.int16)
    slot_a = np.full((NCORES, C_total * P), 200.0, np.float32)
    w_a = np.zeros((NCORES, C_total * P), np.float32)
    idx_a[co, padpos] = (src[order] - starts_a[buck[order]]).astype(np.int16)
    slot_a[co, padpos] = slot[order].astype(np.float32)
    w_a[co, padpos] = w[order]

    # dma_gather index wrap: index i -> [i % 16, i // 16], replicated to 128
    idx16 = idx_a.reshape(NCORES, C_total * 8, 16).transpose(0, 2, 1)
    idx16 = np.ascontiguousarray(
        np.broadcast_to(idx16[:, None, :, :], (NCORES, 8, 16, C_total * 8))
        .reshape(NCORES, P, C_total * 8))
    # per-chunk columns for tensor_scalar scalars (bf16)
    slot_t = np.ascontiguousarray(
        slot_a.reshape(NCORES, C_total, P).transpose(0, 2, 1))
    w_t = np.ascontiguousarray(w_a.reshape(NCORES, C_total, P).transpose(0, 2, 1))

    deg = np.bincount(dst, minlength=N).astype(np.float32)
    n_core = np.minimum(SH, N - np.arange(NCORES) * SH)
    ids = (np.arange(NCORES)[:, None, None] * SH
           + perm[:, :, None] * P + np.arange(P)[None, None, :])  # [NC, NBLK, P]
    valid = (perm[:, :, None] * P
             + np.arange(P)[None, None, :]) < n_core[:, None, None]
    ids_c = np.where(valid, ids, 0)

    xr = np.zeros((NCORES, NBLK, P, D), np.float32)
    xr[valid] = x[ids_c[valid]]
    xr = xr.reshape(NCORES, SHP, D)

    dt = np.zeros((NCORES, NBLK, P), np.float32)
    dt[valid] = deg[ids_c[valid]]
    deg_t = np.ascontiguousarray(dt.transpose(0, 2, 1))     # [NC, 128, NBLK]

    # per-core real-index count per gather call; trailing pads become -1 so
    # the gather ucode skips them (documented negative-tail behavior)
    cnt = np.zeros((NCORES, NCALLS), np.int32)
    sorted_counts = np.take_along_axis(counts, perm[:, :, None], axis=1)
    for i, (s, q) in enumerate(call_sq):
        base = int(K_of[s, q]) * P
        nch = int(NCH4[s, q])
        for c in range(NCORES):
            n_real = int(sorted_counts[c, s, q])
            if n_real == 0:
                n_real = 1          # keep one (idx 0, w=0) row: ucode needs >=1
            cnt[c, i] = n_real
            idx_a[c, base + n_real:base + nch * P] = -1

    # rebuild the wrapped index array with the -1 pads
    idx16 = idx_a.reshape(NCORES, C_total * 8, 16).transpose(0, 2, 1)
    idx16 = np.ascontiguousarray(
        np.broadcast_to(idx16[:, None, :, :], (NCORES, 8, 16, C_total * 8))
        .reshape(NCORES, P, C_total * 8))

    return dict(
        N=N, D=D, SH=SH, NBLK=NBLK, SHP=SHP, starts=starts, C_total=C_total,
        NCH4=NCH4, K_of=K_of, batches=batches, cnt=cnt,
        idx16=idx16, slot_t=slot_t, w_t=w_t,
        xr=xr, deg_t=deg_t, ids=ids, valid=valid,
    )


def _build_program(pre, alpha):
    import concourse.bacc as bacc
    import concourse.bass as bass
    import concourse.tile as tile
    from concourse import mybir

    f32 = mybir.dt.float32
    bf16 = mybir.dt.bfloat16
    eq = mybir.AluOpType.is_equal
    mult = mybir.AluOpType.mult
    mx = mybir.AluOpType.max

    N, NBLK, SHP, starts = pre["N"], pre["NBLK"], pre["SHP"], pre["starts"]
    C_total, NCH4, K_of = pre["C_total"], pre["NCH4"], pre["K_of"]
    batches = pre["batches"]
    NCALLS = pre["cnt"].shape[1]
    Mmax = max(sum(nch for (_, _, _, nch) in gr) for (_, _, _, gr) in batches)

    nc = bacc.Bacc(None, target_bir_lowering=False,
                   dynamic_dma_scratch_size=DMA_SCRATCH)
    x_d = nc.dram_tensor("xh", [N, P], bf16, kind="ExternalInput")
    idx_d = nc.dram_tensor("idx16", [P, C_total * 8], mybir.dt.int16,
                           kind="ExternalInput")
    slot_d = nc.dram_tensor("slot", [P, C_total], f32, kind="ExternalInput")
    wg_d = nc.dram_tensor("wg", [P, C_total], f32, kind="ExternalInput")
    xr_d = nc.dram_tensor("xr", [SHP, P], bf16, kind="ExternalInput")
    deg_d = nc.dram_tensor("deg", [P, NBLK], f32, kind="ExternalInput")
    wt_d = nc.dram_tensor("wt", [P, P], bf16, kind="ExternalInput")
    iota_d = nc.dram_tensor("iota", [P, P], bf16, kind="ExternalInput")
    cnt_d = nc.dram_tensor("cnt", [1, NCALLS], mybir.dt.int32,
                           kind="ExternalInput")
    y_d = nc.dram_tensor("y", [SHP, P], bf16, kind="ExternalOutput")

    with tile.TileContext(nc) as tc:
        with (
            tc.tile_pool(name="const", bufs=1) as cpool,
            tc.tile_pool(name="gather", bufs=3) as gpool,
            tc.tile_pool(name="sw", bufs=8) as swpool,
            tc.tile_pool(name="agg", bufs=3) as aggpool,
            tc.tile_pool(name="xr", bufs=2) as xrpool,
            tc.tile_pool(name="ot", bufs=2) as otpool,
            tc.tile_pool(name="ps1", bufs=2, space="PSUM") as ps1,
            tc.tile_pool(name="ps2", bufs=2, space="PSUM") as ps2,
        ):
            idx_s = cpool.tile([P, C_total * 8], mybir.dt.int16)
            nc.sync.dma_start(out=idx_s[:], in_=idx_d[:, :])
            slot_s = cpool.tile([P, C_total], f32)
            nc.sync.dma_start(out=slot_s[:], in_=slot_d[:, :])
            w_s = cpool.tile([P, C_total], f32)
            nc.sync.dma_start(out=w_s[:], in_=wg_d[:, :])
            wt_s = cpool.tile([P, P], bf16)
            nc.sync.dma_start(out=wt_s[:], in_=wt_d[:, :])
            iota_s = cpool.tile([P, P], bf16)
            nc.sync.dma_start(out=iota_s[:], in_=iota_d[:, :])
            cnt_s = cpool.tile([1, NCALLS], mybir.dt.int32)
            nc.sync.dma_start(out=cnt_s[:], in_=cnt_d[:, :])
            cnt_reg = nc.gpsimd.alloc_register()

            # persistent gather buffers, zeroed once: skipped (negative-idx)
            # tail rows leave stale SBUF data, which must not be NaN/Inf
            gbufs = []
            for _i in range(3):
                gb = cpool.tile([P, Mmax, P], bf16)
                nc.vector.memset(gb[:], 0.0)
                gbufs.append(gb)

            deg_s = cpool.tile([P, NBLK], f32)
            nc.sync.dma_start(out=deg_s[:], in_=deg_d[:, :])
            invd_s = cpool.tile([P, NBLK], f32)
            nc.vector.tensor_scalar(
                out=invd_s[:], in0=deg_s[:], scalar1=1.0, scalar2=None, op0=mx)
            nc.vector.reciprocal(invd_s[:], invd_s[:])
            nc.scalar.mul(invd_s[:], invd_s[:], float(alpha))

            ci_call = 0
            for _rep in range(REPEAT):
              ci_call = 0
              for bi, (s0, s1, gstart, groups) in enumerate(batches):
                nb = s1 - s0
                G = gbufs[bi % 3]
                for (q, s_g, off, nch) in (groups if ABLATE != "no_gather" else []):
                    w0 = starts[q]
                    w1 = min(w0 + 32768, N)
                    i0 = (gstart + off) * P    # global index position
                    nc.gpsimd.reg_load(
                        out_reg=cnt_reg,
                        in_tensor=cnt_s[:, ci_call:ci_call + 1])
                    nc.gpsimd.dma_gather(
                        out_ap=G[:, off:off + nch, :],
                        in_ap=x_d[w0:w1, :],
                        idxs_ap=idx_s[:, i0 // 16:(i0 + nch * P) // 16],
                        num_idxs=nch * P,
                        num_idxs_reg=cnt_reg,
                        elem_size=P,
                    )
                    ci_call += 1

                if ABLATE == "gather_only":
                    continue
                xrt = xrpool.tile([P, nb, P], bf16)
                nc.scalar.dma_start(
                    out=xrt[:],
                    in_=xr_d[s0 * P:s1 * P, :].rearrange(
                        "(nb p) d -> p nb d", p=P),
                )
                ot = otpool.tile([P, nb, P], bf16)

                for s in range(s0, s1):
                    nch = int(NCH4[s].sum())
                    p1 = ps1.tile([P, P], f32)
                    ci = 0
                    for q in range(NBUCK):
                        for c in range(int(NCH4[s, q])):
                            k = int(K_of[s, q]) + c
                            sw = swpool.tile([P, P], bf16)
                            nc.vector.tensor_scalar(
                                out=sw[:], in0=iota_s[:],
                                scalar1=slot_s[:, k:k + 1],
                                scalar2=w_s[:, k:k + 1],
                                op0=eq, op1=mult,
                            )
                            nc.tensor.matmul(
                                p1[:], lhsT=G[:, k - gstart, :], rhs=sw[:],
                                start=(ci == 0), stop=(ci == nch - 1),
                            )
                            ci += 1
                    aggT = aggpool.tile([P, P], bf16)
                    nc.scalar.mul(aggT[:], p1[:], 1.0)
                    p2 = ps2.tile([P, P], f32)
                    nc.tensor.matmul(
                        p2[:], lhsT=aggT[:], rhs=wt_s[:], start=True, stop=True)
                    j = s - s0
                    nc.scalar.mul(ot[:, j, :], p2[:], invd_s[:, s:s + 1])
                    nc.vector.tensor_add(ot[:, j, :], ot[:, j, :], xrt[:, j, :])

                nc.sync.dma_start(
                    out=y_d[s0 * P:s1 * P, :].rearrange(
                        "(nb p) d -> p nb d", p=P),
                    in_=ot[:],
                )

    nc.compile()
    return nc


def kernel(**inputs):
    global LAST_RESULTS
    x = np.ascontiguousarray(np.asarray(inputs["x"], dtype=np.float32))
    ei = np.asarray(inputs["edge_index"])
    w = np.ascontiguousarray(np.asarray(inputs["edge_weight"], dtype=np.float32))
    W = np.asarray(inputs["W"], dtype=np.float32)
    b = np.asarray(inputs["b"], dtype=np.float32)
    alpha = float(np.asarray(inputs["alpha"]))
    src = ei[0].astype(np.int64)
    dst = ei[1].astype(np.int64)

    pre = _preprocess(x, src, dst, w)
    N, D = pre["N"], pre["D"]
    assert D == P

    nc = _build_program(pre, alpha)

    xh = np.ascontiguousarray(x.astype(bfloat16))
    wt = np.ascontiguousarray(W.T.astype(bfloat16))
    iota = np.ascontiguousarray(
        np.broadcast_to(np.arange(P, dtype=np.float32), (P, P))).astype(bfloat16)
    # residual with bias pre-folded: xr' = x[ids] + alpha*b
    xr2 = pre["xr"] + (alpha * b.astype(np.float32))[None, None, :]

    in_maps = []
    for c in range(NCORES):
        in_maps.append({
            "xh": xh,
            "idx16": pre["idx16"][c],
            "slot": pre["slot_t"][c],
            "wg": pre["w_t"][c],
            "xr": np.ascontiguousarray(xr2[c].astype(bfloat16)),
            "deg": pre["deg_t"][c],
            "wt": wt,
            "iota": iota,
        })

    global LAST_NC, LAST_IN_MAPS, LAST_PRE
    LAST_NC, LAST_IN_MAPS, LAST_PRE = nc, in_maps, pre

    from concourse.bass_utils import run_bass_kernel_spmd
    kw = {"trace": True} if TRACE else {}
    res = run_bass_kernel_spmd(
        nc, in_maps, core_ids=list(range(NCORES)), **kw)
    LAST_RESULTS = res

    out = np.empty((N, P), np.float32)
    NBLK = pre["NBLK"]
    valid = pre["valid"]
    ids = pre["ids"]
    for c in range(NCORES):
        y = np.asarray(res.results[c]["y"]).astype(np.float32).reshape(NBLK, P, P)
        out[ids[c][valid[c]]] = y[valid[c]]
    return out
